# revision 3
# baseline (speedup 1.0000x reference)
"""DelayRNN Trainium2 kernel — weights-stationary T-layout version.

Sharding: data-parallel over batch, 4 rows/core on 8 cores (as baseline).

Architecture (v2, replaces the moving-weights + per-step-transpose design):
  All per-step GEMMs run with the WEIGHT chunk as the PE stationary operand
  (lhsT = W[kc-chunk, mc-chunk], rhs = h^T chunk [128, BL]); outputs land
  directly in transposed layout [feature-part, kc, b], so the per-step
  T1/T2 transposes and all FD=512 PSUM copies disappear.  Everything in the
  recurrence lives in T layout [p, kc, b] with feature = kc*128 + p.

  Weights in bf16 (FWL halves LDWEIGHTS); h quantized to bf16 per step
  (validated offline: rel err ~5e-3 vs 2e-2 budget).
    Wd = Wh@W_pass - Wh ;  enc: h' = h0@Wh + (m*h0)@Wd + cmix
    dec: h' = h0@Wp2
  cmix^T for all (b,t) is computed on-device at setup and stays SBUF-resident
  (bf16), so the recurrence runs with ZERO DMA traffic.

  HAM: small dummy matmuls (scratch PSUM bank) are interleaved in PE program
  order after each GEMM phase; they execute during the elementwise-tail gaps
  and keep the PE activity monitor at full clock (the baseline lost ~1.9us
  per step to K=4/8 re-throttling).
"""

import sys
import numpy as np

for _p in ("/opt/trn_rl_repo",):
    if _p not in sys.path:
        sys.path.append(_p)

from contextlib import ExitStack

import concourse.bass as bass
import concourse.tile as tile
from concourse import bacc, mybir
from concourse.masks import make_identity

FP32 = mybir.dt.float32
FP32R = mybir.dt.float32r
BF16 = mybir.dt.bfloat16
FP8E3 = mybir.dt.float8e3
I32 = mybir.dt.int32

B, S, I, H, C = 32, 256, 128, 512, 64
T_OUT = 64
NCORES = 8
BL = B // NCORES        # 4 batch rows per core
KC = H // 128           # 4 feature chunks
D = 16                  # delay slots 1..16
CB = KC * BL

Sig = mybir.ActivationFunctionType.Sigmoid
Abs = mybir.ActivationFunctionType.Abs
Op = mybir.AluOpType


def build(seq_len=S, t_out=T_OUT, zero_bias=True, l2_fp8=False, debug=False):
    nc = bacc.Bacc("TRN2", target_bir_lowering=False, debug=False)
    NROW = BL * seq_len
    NMT = NROW // 128
    L2DT = FP8E3 if l2_fp8 else BF16
    WSCALE = 32.0 if l2_fp8 else 1.0
    total_steps_dbg = seq_len + t_out
    if debug:
        dbg_cmix = nc.dram_tensor("dbg_cmix", [128, KC, NROW], BF16,
                                  kind="ExternalOutput")
        dbg_hb = nc.dram_tensor("dbg_hb", [total_steps_dbg, 128, KC, BL],
                                BF16, kind="ExternalOutput")
        dbg_sig = nc.dram_tensor("dbg_sig", [total_steps_dbg, 128, 2, KC, BL],
                                 FP32, kind="ExternalOutput")
        dbg_hTf = nc.dram_tensor("dbg_hTf", [total_steps_dbg, 128, KC, BL],
                                 BF16, kind="ExternalOutput")

    # ---------------- DRAM I/O ----------------
    dx = nc.dram_tensor("x", [BL, seq_len, I], FP32, kind="ExternalInput")
    dlen = nc.dram_tensor("lengths", [BL], I32, kind="ExternalInput")
    dwin = nc.dram_tensor("W_in", [I + H, H], FP32, kind="ExternalInput")
    dwpass = nc.dram_tensor("W_pass", [H, H], FP32, kind="ExternalInput")
    dwtau = nc.dram_tensor("W_tau", [H, H], FP32, kind="ExternalInput")
    dwmem = nc.dram_tensor("W_mem", [H, H], FP32, kind="ExternalInput")
    dwout = nc.dram_tensor("W_out", [H, C], FP32, kind="ExternalInput")
    dbias = {}
    for nm, ln in [("b_in", H), ("b_pass", H), ("b_tau", H),
                   ("b_mem", H), ("b_out", C)]:
        dbias[nm] = nc.dram_tensor(nm, [ln], FP32, kind="ExternalInput")
    dout = nc.dram_tensor("out", [BL, t_out, C], FP32, kind="ExternalOutput")

    with tile.TileContext(nc) as tc, ExitStack() as ctx:
        persist = ctx.enter_context(tc.tile_pool(name="persist", bufs=1))

        # ------------- persistent SBUF tensors -------------
        whb = persist.tile([128, KC, H], BF16, name="whb")
        wdb = persist.tile([128, KC, H], BF16, name="wdb")
        wpb = persist.tile([128, KC, H], BF16, name="wpb")
        wtaub = persist.tile([128, KC, H], L2DT, name="wtaub")
        wmemb = persist.tile([128, KC, H], L2DT, name="wmemb")
        woutb = persist.tile([128, KC, C], BF16, name="woutb")
        iota16 = persist.tile([128, D], FP32, name="iota16")   # 1..16
        maskRb = persist.tile([128, seq_len, BL], BF16, name="maskRb")
        cmixT = persist.tile([128, KC, NROW], BF16, name="cmixT")
        buf0 = persist.tile([128, D, CB], FP32, name="buf0")
        buf1 = persist.tile([128, D, CB], FP32, name="buf1")
        h0coll = persist.tile([128, KC, t_out, BL], BF16, name="h0coll")
        out_sbs = [persist.tile([128, C], FP32, name=f"out_sb{i}")
                   for i in range((t_out * BL + 127) // 128)]
        if not zero_bias:
            tmbT = persist.tile([128, 2, KC], FP32, name="tmbT")
            cdecT = persist.tile([128, KC, 1], FP32, name="cdecT")
            b_out_r = persist.tile([128, C], FP32, name="b_out_r")

        # ------------- setup (scoped pools) -------------
        with tc.tile_pool(name="setup_ps", bufs=2, space="PSUM") as setup_ps, \
                tc.tile_pool(name="setup_sb", bufs=1) as setup_sb:
            # raw fp32 weight loads, [kp, kc, n] convention (k = kc*128+kp)
            wh_d = setup_sb.tile([128, KC, H], FP32, name="wh_d")
            wpass_d = setup_sb.tile([128, KC, H], FP32, name="wpass_d")
            wtau_d = setup_sb.tile([128, KC, H], FP32, name="wtau_d")
            wmem_d = setup_sb.tile([128, KC, H], FP32, name="wmem_d")
            wx_d = setup_sb.tile([128, H], FP32, name="wx_d")
            wout_d = setup_sb.tile([128, KC, C], FP32, name="wout_d")
            nc.sync.dma_start(wh_d[:], dwin[:H].rearrange(
                "(kc kp) n -> kp kc n", kp=128))
            nc.sync.dma_start(wx_d[:], dwin[H:])
            nc.sync.dma_start(wpass_d[:], dwpass[:].rearrange(
                "(kc kp) n -> kp kc n", kp=128))
            nc.sync.dma_start(wtau_d[:], dwtau[:].rearrange(
                "(kc kp) n -> kp kc n", kp=128))
            nc.sync.dma_start(wmem_d[:], dwmem[:].rearrange(
                "(kc kp) n -> kp kc n", kp=128))
            nc.sync.dma_start(wout_d[:], dwout[:].rearrange(
                "(kc kp) n -> kp kc n", kp=128))

            # fp32r copies for the setup GEMMs
            wpass_r = setup_sb.tile([128, KC, H], FP32R, name="wpass_r")
            wx_r = setup_sb.tile([128, H], FP32R, name="wx_r")
            nc.vector.tensor_copy(wpass_r[:], wpass_d[:])
            nc.vector.tensor_copy(wx_r[:], wx_d[:])

            id128 = setup_sb.tile([128, 128], FP32, name="id128")
            make_identity(nc, id128[:])

            iota16_i = setup_sb.tile([128, D], I32, name="iota16_i")
            nc.gpsimd.iota(iota16_i[:], pattern=[[1, D]], base=1,
                           channel_multiplier=0)
            nc.vector.tensor_copy(iota16[:], iota16_i[:])

            # masks: maskRb[p, t, b] = (t < len[b]) in bf16,
            #        mk_row_i[p, r] = (r%S < len[r//S]) int for cmix select
            iota_t = setup_sb.tile([128, seq_len], I32, name="iota_t")
            nc.gpsimd.iota(iota_t[:], pattern=[[1, seq_len]], base=0,
                           channel_multiplier=0)
            lenR = setup_sb.tile([128, BL], I32, name="lenR")
            nc.sync.dma_start(
                lenR[:], dlen[:].unsqueeze(0).to_broadcast([128, BL]))
            mkR_i = setup_sb.tile([128, seq_len, BL], I32, name="mkR_i")
            nc.vector.tensor_tensor(
                out=mkR_i[:],
                in0=iota_t[:].unsqueeze(2).to_broadcast([128, seq_len, BL]),
                in1=lenR[:].unsqueeze(1).to_broadcast([128, seq_len, BL]),
                op=Op.is_lt)
            nc.vector.tensor_copy(maskRb[:], mkR_i[:])
            mk_row_i = setup_sb.tile([128, BL, seq_len], I32, name="mk_row_i")
            nc.vector.tensor_tensor(
                out=mk_row_i[:],
                in0=iota_t[:].unsqueeze(1).to_broadcast([128, BL, seq_len]),
                in1=lenR[:].unsqueeze(2).to_broadcast([128, BL, seq_len]),
                op=Op.is_lt)

            # WhT / WxT via PE transposes (for Wp2 / Wxp products)
            whT = setup_sb.tile([128, KC, H], FP32R, name="whT")
            wxT = setup_sb.tile([128, KC, I], FP32R, name="wxT")
            for jc in range(KC):
                for kc in range(KC):
                    pst = setup_ps.tile([128, 128], FP32, tag="setup_T")
                    nc.tensor.transpose(pst[:], wh_d[:, kc, bass.ts(jc, 128)],
                                        id128[:])
                    nc.vector.tensor_copy(whT[:, jc, bass.ts(kc, 128)],
                                          pst[:])
            for jc in range(KC):
                pst = setup_ps.tile([128, 128], FP32, tag="setup_T")
                nc.tensor.transpose(pst[:], wx_d[:, bass.ts(jc, 128)],
                                    id128[:])
                nc.vector.tensor_copy(wxT[:, jc, :], pst[:])

            # Wp2 = Wh @ W_pass (fp32), Wxp = Wx @ W_pass
            wp2_f = setup_sb.tile([128, KC, H], FP32, name="wp2_f")
            wxp = setup_sb.tile([128, H], FP32R, name="wxp")
            for m in range(KC):
                psg = setup_ps.tile([128, H], FP32, tag="setup_G")
                for jc in range(KC):
                    nc.tensor.matmul(psg[:], whT[:, jc, bass.ts(m, 128)],
                                     wpass_r[:, jc, :],
                                     start=(jc == 0), stop=(jc == KC - 1))
                nc.vector.tensor_copy(wp2_f[:, m, :], psg[:])
            psg = setup_ps.tile([128, H], FP32, tag="setup_G")
            for jc in range(KC):
                nc.tensor.matmul(psg[:], wxT[:, jc, :], wpass_r[:, jc, :],
                                 start=(jc == 0), stop=(jc == KC - 1))
            nc.vector.tensor_copy(wxp[:], psg[:])

            # quantized weight casts
            wd_f = setup_sb.tile([128, KC, H], FP32, name="wd_f")
            nc.vector.tensor_tensor(out=wd_f[:], in0=wp2_f[:], in1=wh_d[:],
                                    op=Op.subtract)
            nc.vector.tensor_copy(whb[:], wh_d[:])
            nc.vector.tensor_copy(wdb[:], wd_f[:])
            nc.vector.tensor_copy(wpb[:], wp2_f[:])
            if l2_fp8:
                nc.vector.tensor_scalar(out=wtaub[:], in0=wtau_d[:],
                                        scalar1=WSCALE, scalar2=None,
                                        op0=Op.mult)
                nc.vector.tensor_scalar(out=wmemb[:], in0=wmem_d[:],
                                        scalar1=WSCALE, scalar2=None,
                                        op0=Op.mult)
            else:
                nc.vector.tensor_copy(wtaub[:], wtau_d[:])
                nc.vector.tensor_copy(wmemb[:], wmem_d[:])
            nc.vector.tensor_copy(woutb[:], wout_d[:])

            # biases (T layout, per (p, kc))
            if not zero_bias:
                nc.sync.dma_start(
                    tmbT[:, 0, :], dbias["b_tau"][:].rearrange(
                        "(c p) -> p c", p=128))
                nc.sync.dma_start(
                    tmbT[:, 1, :], dbias["b_mem"][:].rearrange(
                        "(c p) -> p c", p=128))
                nc.sync.dma_start(
                    b_out_r[:], dbias["b_out"][:].unsqueeze(0)
                    .to_broadcast([128, C]))
                # cdec = b_in @ W_pass + b_pass, in T layout [p, kc]
                binT = setup_sb.tile([128, KC, 1], FP32, name="binT")
                nc.sync.dma_start(binT[:], dbias["b_in"][:].rearrange(
                    "(c p) -> p c", p=128).unsqueeze(2))
                binRow = setup_sb.tile([128, H], FP32, name="binRow")
                nc.sync.dma_start(
                    binRow[:], dbias["b_in"][:].unsqueeze(0)
                    .to_broadcast([128, H]))
                binRow_r = setup_sb.tile([1, H], FP32R, name="binRow_r")
                nc.vector.tensor_copy(binRow_r[:], binRow[0:1, :])
                psd = setup_ps.tile([1, H], FP32, tag="setup_D")
                # (1,H) @ (H,H): lhsT = binT chunks? use row vector:
                # out[1, H] = binRow_r[1(K?)...] -- instead compute via
                # lhsT=[K=128,1] chunks of b_in^T against wpass
                psd2 = setup_ps.tile([1, H], FP32, tag="setup_D2")
                for ccc in range(KC):
                    nc.tensor.matmul(psd2[:],
                                     binT[:, ccc, :].bitcast(FP32),
                                     wpass_r[:, ccc, :],
                                     start=(ccc == 0), stop=(ccc == KC - 1))
                bps = setup_sb.tile([1, H], FP32, name="bps")
                nc.sync.dma_start(bps[:], dbias["b_pass"][:].unsqueeze(0))
                cdec_row = setup_sb.tile([1, H], FP32, name="cdec_row")
                nc.vector.tensor_tensor(out=cdec_row[:], in0=psd2[:],
                                        in1=bps[:], op=Op.add)
                # scatter row [1, H] -> T layout [128, KC] via DRAM bounce
                dcdec = nc.dram_tensor("cdec_scratch", [H], FP32)
                nc.sync.dma_start(dcdec[:], cdec_row[:].squeeze(0))
                nc.sync.dma_start(cdecT[:], dcdec[:].rearrange(
                    "(c p) -> p c", p=128).unsqueeze(2))
                del psd

            # x -> xT, then cxT/cpT GEMMs (weights-stationary, T output)
            x_sb = setup_sb.tile([128, NMT, I], FP32, name="x_sb")
            xT = setup_sb.tile([128, NMT, 128], FP32R, name="xT")
            nc.sync.dma_start(
                x_sb[:],
                dx[:].rearrange("b t i -> (b t) i").rearrange(
                    "(m p) i -> p m i", p=128))
            for m in range(NMT):
                pst = setup_ps.tile([128, 128], FP32, tag="setup_T")
                nc.tensor.transpose(pst[:], x_sb[:, m, :], id128[:])
                nc.vector.tensor_copy(xT[:, m, :], pst[:])

            # cxT/cpT: lhsT = Wx[K=I=128, M=H-chunk] (= wx_r cols), rhs = xT
            cpTb = setup_sb.tile([128, KC, NROW], BF16, name="cpTb")
            MG = NMT // 4  # groups of 4 m-tiles -> N=512 matmuls
            for hc in range(KC):
                for mg in range(MG):
                    psx = setup_ps.tile([128, 512], FP32, tag="setup_CX")
                    nc.tensor.matmul(
                        psx[:], wx_r[:, bass.ts(hc, 128)],
                        xT[:, 4 * mg:4 * (mg + 1), :],
                        start=True, stop=True)
                    if zero_bias:
                        nc.vector.tensor_copy(
                            cmixT[:, hc, bass.ts(mg, 512)], psx[:])
                    else:
                        nc.vector.tensor_tensor(
                            out=cmixT[:, hc, bass.ts(mg, 512)],
                            in0=psx[:],
                            in1=binT[:, hc, :].to_broadcast([128, 512]),
                            op=Op.add)
                    psx2 = setup_ps.tile([128, 512], FP32, tag="setup_CX")
                    nc.tensor.matmul(
                        psx2[:], wxp[:, bass.ts(hc, 128)],
                        xT[:, 4 * mg:4 * (mg + 1), :],
                        start=True, stop=True)
                    if zero_bias:
                        nc.vector.tensor_copy(
                            cpTb[:, hc, bass.ts(mg, 512)], psx2[:])
                    else:
                        nc.vector.tensor_tensor(
                            out=cpTb[:, hc, bass.ts(mg, 512)],
                            in0=psx2[:],
                            in1=cdecT[:, hc, :].to_broadcast([128, 512]),
                            op=Op.add)
            # cmixT = where(mask_row, cpT, cxT); mask over rows r=(b,t)
            for hc in range(KC):
                nc.vector.copy_predicated(
                    out=cmixT[:, hc, :].bitcast(mybir.dt.uint16),
                    mask=mk_row_i[:].rearrange("p b t -> p (b t)"),
                    data=cpTb[:, hc, :].bitcast(mybir.dt.uint16))
            if debug:
                nc.sync.dma_start(dbg_cmix[:], cmixT[:])

        # all setup work (and its SBUF/PSUM pool space) must retire before
        # the loop pools reuse the memory
        tc.strict_bb_all_engine_barrier()

        # ------------- main recurrence -------------
        psum = ctx.enter_context(tc.tile_pool(name="mn_ps", bufs=1,
                                              space="PSUM"))
        loop_sb = ctx.enter_context(tc.tile_pool(name="mn_sb", bufs=2))

        bufs = [buf0, buf1]
        hb = loop_sb.tile([128, KC, BL], BF16, tag="hb")
        nc.vector.memset(hb[:].bitcast(mybir.dt.uint16), 0)
        nc.gpsimd.memset(buf0[:], 0.0)

        deferred = []
        buf_idx = 0
        total_steps = seq_len + t_out

        # one-time HAM warm burst: ~40 dense N=512 matmuls (rotating over 4
        # scratch banks so WAW deps don't serialize them) push the PE
        # activity monitor to K=8/8; the loop's gaps are all well under the
        # ~3.4us idle window, so it stays warm from here on.
        for _ in range(40):
            pd = psum.tile([128, 512], FP32, tag="ps_dum", bufs=2)
            nc.tensor.matmul(pd[:], whb[:, 0, 0:128], whb[:, 0, :],
                             start=True, stop=True)

        for t in range(total_steps):
            is_enc = t < seq_len
            td = t - seq_len
            last = (t == total_steps - 1)

            if not is_enc:
                nc.vector.tensor_copy(h0coll[:, :, td, :], hb[:])
                if last:
                    break

            if is_enc:
                mb = loop_sb.tile([128, KC, BL], BF16, tag="mb")
                nc.vector.tensor_tensor(
                    out=mb[:], in0=hb[:],
                    in1=maskRb[:, t, :].unsqueeze(1)
                    .to_broadcast([128, KC, BL]),
                    op=Op.mult)

            # deferred buffer update from the previous step
            while deferred:
                deferred.pop(0)()

            # L1: ps_h[p, mc, b] = sum_kc WhT-chunks @ hb (+ Wd @ mb)
            ps_h = psum.tile([128, KC, BL], FP32, tag="ps_h")
            if is_enc:
                for mc in range(KC):
                    for kc in range(KC):
                        nc.tensor.matmul(
                            ps_h[:, mc, :], whb[:, kc, bass.ts(mc, 128)],
                            hb[:, kc, :], start=(kc == 0), stop=False)
                    for kc in range(KC):
                        nc.tensor.matmul(
                            ps_h[:, mc, :], wdb[:, kc, bass.ts(mc, 128)],
                            mb[:, kc, :], start=False, stop=(kc == KC - 1))
            else:
                for mc in range(KC):
                    for kc in range(KC):
                        nc.tensor.matmul(
                            ps_h[:, mc, :], wpb[:, kc, bass.ts(mc, 128)],
                            hb[:, kc, :], start=(kc == 0),
                            stop=(kc == KC - 1))

            # h'T in bf16 (feeds both L2 rhs and q)
            hTf = loop_sb.tile([128, KC, BL], BF16, tag="hTf")
            if is_enc:
                cm = cmixT[:].rearrange(
                    "p c (b t) -> p c b t", b=BL)[:, :, :, t]
                nc.vector.tensor_tensor(out=hTf[:], in0=ps_h[:], in1=cm,
                                        op=Op.add)
            elif not zero_bias:
                nc.vector.tensor_tensor(
                    out=hTf[:], in0=ps_h[:],
                    in1=cdecT[:].to_broadcast([128, KC, BL]), op=Op.add)
            else:
                nc.vector.tensor_copy(hTf[:], ps_h[:])
            hb2 = hTf
            if debug:
                nc.sync.dma_start(dbg_hTf[t], hTf[:])

            # L2: tau and mem GEMMs into SEPARATE full PSUM banks so the tau
            # sigmoid + reciprocal (Scalar/DVE) can overlap the mem GEMMs
            # (same-bank PE-write + Scalar-read would collide / serialize).
            ps_tau = psum.tile([128, 512], FP32, tag="ps_tau")
            ps_mem = psum.tile([128, 512], FP32, tag="ps_mem")
            ps_tau_v = ps_tau[:, 0:CB].rearrange("p (c b) -> p c b", c=KC)
            ps_mem_v = ps_mem[:, 0:CB].rearrange("p (c b) -> p c b", c=KC)
            for mc in range(KC):
                for kc in range(KC):
                    nc.tensor.matmul(
                        ps_tau_v[:, mc, :], wtaub[:, kc, bass.ts(mc, 128)],
                        hb2[:, kc, :], start=(kc == 0), stop=(kc == KC - 1))
            sig0 = loop_sb.tile([128, KC, BL], FP32, tag="sig0")
            if zero_bias:
                nc.scalar.activation(sig0[:], ps_tau_v, Sig,
                                     scale=1.0 / WSCALE)
            else:
                tml = loop_sb.tile([128, KC, BL], FP32, tag="tml")
                nc.vector.scalar_tensor_tensor(
                    out=tml[:], in0=ps_tau_v, scalar=1.0 / WSCALE,
                    in1=tmbT[:, 0].unsqueeze(2).to_broadcast([128, KC, BL]),
                    op0=Op.mult, op1=Op.add)
                nc.scalar.activation(sig0[:], tml[:], Sig)
            taup = loop_sb.tile([128, KC, BL], FP32, tag="taup")
            nc.vector.tensor_scalar(out=taup[:], in0=sig0[:], scalar1=16.0,
                                    scalar2=1.0, op0=Op.mult, op1=Op.max)
            rtau = loop_sb.tile([128, KC, BL], FP32, tag="rtau")
            nc.vector.reciprocal(out=rtau[:], in_=taup[:])

            for mc in range(KC):
                for kc in range(KC):
                    nc.tensor.matmul(
                        ps_mem_v[:, mc, :], wmemb[:, kc, bass.ts(mc, 128)],
                        hb2[:, kc, :], start=(kc == 0), stop=(kc == KC - 1))
            sig1 = loop_sb.tile([128, KC, BL], FP32, tag="sig1")
            if zero_bias:
                nc.scalar.activation(sig1[:], ps_mem_v, Sig,
                                     scale=1.0 / WSCALE)
            else:
                tml1 = loop_sb.tile([128, KC, BL], FP32, tag="tml1")
                nc.vector.scalar_tensor_tensor(
                    out=tml1[:], in0=ps_mem_v, scalar=1.0 / WSCALE,
                    in1=tmbT[:, 1].unsqueeze(2).to_broadcast([128, KC, BL]),
                    op0=Op.mult, op1=Op.add)
                nc.scalar.activation(sig1[:], tml1[:], Sig)

            q = loop_sb.tile([128, KC, BL], FP32, tag="q")
            nc.vector.tensor_tensor(out=q[:], in0=sig1[:], in1=hTf[:],
                                    op=Op.mult)
            t1 = loop_sb.tile([128, KC, BL], FP32, tag="t1")
            nc.vector.tensor_tensor(out=t1[:], in0=q[:], in1=rtau[:],
                                    op=Op.mult)
            bcur, bnxt = bufs[buf_idx], bufs[buf_idx ^ 1]
            buf_idx ^= 1
            hb = loop_sb.tile([128, KC, BL], BF16, tag="hb")
            nc.vector.tensor_tensor(
                out=hb[:],
                in0=bcur[:, 0, :].rearrange("p (c b) -> p c b", c=KC),
                in1=t1[:], op=Op.add)
            if debug:
                nc.sync.dma_start(dbg_sig[t, :, 0], sig0[:])
                nc.sync.dma_start(dbg_sig[t, :, 1], sig1[:])
                nc.sync.dma_start(dbg_hb[t], hb[:])

            # deferred: r_d weights + buffer shift-add (runs during next L1)
            if t < total_steps - 2:
                def make_deferred(taup=taup, q=q, bcur=bcur, bnxt=bnxt):
                    def emit():
                        tpv = taup[:].rearrange("p c b -> p (c b)")
                        qv = q[:].rearrange("p c b -> p (c b)")
                        wt = loop_sb.tile([128, D, CB], FP32, tag="wt")
                        nc.gpsimd.tensor_tensor(
                            out=wt[:],
                            in0=tpv.unsqueeze(1).to_broadcast([128, D, CB]),
                            in1=iota16[:].unsqueeze(2).to_broadcast(
                                [128, D, CB]),
                            op=Op.subtract)
                        nc.scalar.activation(wt[:], wt[:], Abs)
                        nc.scalar.add(wt[:], wt[:], 1.0)
                        wr = loop_sb.tile([128, D, CB], FP32, tag="wr")
                        ws = loop_sb.tile([128, D, CB], FP32, tag="ws")
                        nc.vector.reciprocal_approx_accurate(
                            out=wr[:], in_=wt[:], scratch=ws[:])
                        nc.vector.tensor_tensor(
                            out=wr[:], in0=wr[:],
                            in1=qv.unsqueeze(1).to_broadcast([128, D, CB]),
                            op=Op.mult)
                        nc.gpsimd.tensor_tensor(
                            out=bnxt[:, 0:D - 1, :], in0=bcur[:, 1:D, :],
                            in1=wr[:, 1:D, :], op=Op.add)
                        nc.gpsimd.memset(bnxt[:, D - 1, :], 0.0)
                    return emit
                deferred.append(make_deferred())

        # batched output GEMM: out = h0coll @ W_out (+ b_out)
        rows = t_out * BL
        dout_tb = dout[:].transpose([1, 0, 2])      # [t, b, c]
        for start in range(0, rows, 128):
            mrows = min(128, rows - start)
            t0, tn = start // BL, mrows // BL
            ps_o = psum.tile([mrows, C], FP32, tag="ps_o")
            for c in range(KC):
                nc.tensor.matmul(
                    ps_o[:], h0coll[:, c, t0:t0 + tn, :], woutb[:, c, :],
                    start=(c == 0), stop=(c == KC - 1))
            out_sb = out_sbs[start // 128]
            if zero_bias:
                nc.vector.tensor_copy(out_sb[0:mrows, :], ps_o[:])
            else:
                nc.vector.tensor_tensor(out=out_sb[0:mrows, :], in0=ps_o[:],
                                        in1=b_out_r[0:mrows, :], op=Op.add)
            for tt in range(tn):
                nc.sync.dma_start(dout[:, t0 + tt, :],
                                  out_sb[bass.ts(tt, BL), :])

    nc.compile()
    return nc


_CACHE = {}


def _get_module(seq_len, t_out, zero_bias):
    key = (seq_len, t_out, zero_bias)
    if key not in _CACHE:
        _CACHE[key] = build(seq_len, t_out, zero_bias)
    return _CACHE[key]


def kernel(**inputs):
    x = np.ascontiguousarray(np.asarray(inputs["x"], dtype=np.float32))
    lengths = np.ascontiguousarray(
        np.asarray(inputs["lengths"]).astype(np.int32))
    t_out = int(inputs["out_lengths"])
    seq_len = x.shape[1]
    names = ["W_in", "W_pass", "W_tau", "W_mem", "W_out",
             "b_in", "b_pass", "b_tau", "b_mem", "b_out"]
    warrs = {n: np.ascontiguousarray(np.asarray(inputs[n], dtype=np.float32))
             for n in names}
    zero_bias = all(not np.any(warrs[n]) for n in
                    ["b_in", "b_pass", "b_tau", "b_mem", "b_out"])
    nc = _get_module(seq_len, t_out, zero_bias)

    from concourse import bass_utils
    in_maps = []
    for c in range(NCORES):
        sl = slice(c * BL, (c + 1) * BL)
        m = {"x": x[sl], "lengths": lengths[sl]}
        m.update(warrs)
        in_maps.append(m)
    res = bass_utils.run_bass_kernel_spmd(
        nc, in_maps, core_ids=list(range(NCORES)))
    out = np.concatenate([res.results[c]["out"] for c in range(NCORES)],
                         axis=0)
    return out
